# revision 21
# baseline (speedup 1.0000x reference)
"""Trainium2 Bass kernel for causal GQA self-attention (B=2, T=2048, C=2048,
Hq=16, Hkv=4, d=128, RoPE base 1e6).

Sharding: 8 cores = 2 batches x 4 kv-head groups. Each core computes, for its
(batch b, kv group g): the q/k/v projections restricted to that group (4 q
heads + 1 kv head), RoPE, causal attention, and the partial o_proj
(y_group @ Wo[group rows]). The host sums the 4 partial o_proj outputs per
batch (the all-reduce/unshard step of tensor parallelism).

Device layout notes:
  - x is shipped pre-transposed (xT = x[b].T) and pre-cast to bf16 so the
    contraction dim (C) lands on SBUF partitions for all projection matmuls.
  - q/k are produced transposed (qT/kT = [d, T]); scores are computed
    transposed (S^T = k @ qT, [k, q]) so softmax exp needs no cross-partition
    work and P^T feeds the AV matmul directly (yT = v^T @ P^T), no transposes.
  - projections run in slice-pairs with the contraction (ci) loop outermost
    for the first pair so compute streams behind the xT DMAs; later pairs run
    t4-outer so each chunk's bias+rope tail overlaps remaining matmuls
    instead of bursting at the end.
  - input DMAs are ordered small-constants first, then (wk[ci], xT[ci])
    interleaved, then wv/wq/wo, so the first projection matmul can start
    ~4us in instead of waiting behind bulk weight traffic.
  - causal masking is structural: score/AV/rowsum matmuls on diagonal-block
    groups are width-trimmed to the causal region (the exp still covers the
    full tile; the dead columns are never read downstream).
  - softmax skips max-subtraction (scores are O(1); exp cannot overflow) and
    normalizes late: per-(h,qc) rowsums via a trimmed ones-vector matmul,
    then the [1,512] rowsum row is broadcast to [128,512] with a K=1
    fp32r ones-column matmul (no DRAM round-trip), 1/s = exp(-ln(s)) on ACT
    over the broadcast tile, applied to an eagerly-evicted unnormalized yT.
  - the v bias is folded out entirely: since softmax rows sum to 1, bv
    contributes the constant row bv_tiled @ Wo_g, added on the host.
  - o_proj is interleaved into the attention loop so PE has fill work during
    each chunk's normalization chain; q-chunks run largest-first so the
    small-chunk tails overlap accumulated o_proj work.
"""

import numpy as np
import ml_dtypes

import concourse.bass as bass
import concourse.mybir as mybir
from concourse import bacc
from concourse.tile import TileContext
from concourse.bass_utils import run_bass_kernel_spmd
from concourse.masks import make_identity

BF16 = mybir.dt.bfloat16
F32 = mybir.dt.float32
F32R = mybir.dt.float32r

T = 2048
C = 2048
D = 128
NH = 4           # q heads per core
CI = C // 128    # contraction chunks
TC = T // 512    # t chunks of 512
TB = T // 128    # t blocks of 128
SCALE = 1.0 / np.sqrt(D)

_PROGRAM = None


def _ts(i, s):
    return bass.ts(i, s)


def _patch_act_tables():
    """Force every ACT function this kernel uses to resolve to the
    natural_log_exp_and_others table set, so the whole kernel needs exactly
    one ACT_TABLE_LOAD (the default chooser alternates exp_and_others /
    natural_log_exp_and_others, costing ~1.3us per switch, dozens of times).
    Returns an undo callable."""
    import concourse.bacc as bacc_mod

    orig = bacc_mod.get_activation_tables
    A = mybir.ActivationFunctionType
    mine = {A.Exp, A.Ln, A.Identity, A.Copy}

    def patched(arch):
        tables = dict(orig(arch))
        for name in tables:
            if name != "natural_log_exp_and_others":
                tables[name] = set(tables[name]) - mine
        return tables

    bacc_mod.get_activation_tables = patched

    def undo():
        bacc_mod.get_activation_tables = orig

    return undo


def _build_program():
    undo = _patch_act_tables()
    try:
        return _build_program_inner()
    finally:
        undo()


def _build_program_inner():
    nc = bacc.Bacc("TRN2", target_bir_lowering=False, debug=False, num_devices=8)

    xT_d = nc.dram_tensor("xT", [C, T], BF16, kind="ExternalInput").ap()
    wq_d = nc.dram_tensor("wq", [C, NH * D], BF16, kind="ExternalInput").ap()
    wk_d = nc.dram_tensor("wk", [C, D], BF16, kind="ExternalInput").ap()
    wv_d = nc.dram_tensor("wv", [C, D], BF16, kind="ExternalInput").ap()
    wo_d = nc.dram_tensor("wo", [NH * D, C], BF16, kind="ExternalInput").ap()
    bq_d = nc.dram_tensor("bq", [D, NH], F32, kind="ExternalInput").ap()
    bk_d = nc.dram_tensor("bk", [D, 1], F32, kind="ExternalInput").ap()
    cos_d = nc.dram_tensor("cosT", [D, T], F32, kind="ExternalInput").ap()
    sin_d = nc.dram_tensor("sinT", [D, T], F32, kind="ExternalInput").ap()
    tri_d = nc.dram_tensor("tri", [D, D], BF16, kind="ExternalInput").ap()
    out_d = nc.dram_tensor("out", [T, C], F32, kind="ExternalOutput").ap()

    Ident = mybir.ActivationFunctionType.Identity
    Exp = mybir.ActivationFunctionType.Exp
    Log = mybir.ActivationFunctionType.Ln

    with TileContext(nc) as tc:
        with (
            tc.tile_pool(name="consts", bufs=1) as consts,
            tc.tile_pool(name="acts", bufs=1) as acts,
        ):
            # ---- resident constants (DMAs emitted in criticality order) ---
            bq_sb = consts.tile([128, NH], F32)
            nc.sync.dma_start(out=bq_sb[:], in_=bq_d[:])
            bk_sb = consts.tile([128, 1], F32)
            nc.sync.dma_start(out=bk_sb[:], in_=bk_d[:])
            cos_sb = consts.tile([128, T], F32)
            sin_sb = consts.tile([128, T], F32)
            tri_sb = consts.tile([128, 128], BF16)
            nc.sync.dma_start(out=tri_sb[:], in_=tri_d[:])
            ones_sb = consts.tile([128, 1], BF16)
            nc.vector.memset(ones_sb[:], 1.0)
            onesc_f32 = consts.tile([1, 128], F32)
            nc.vector.memset(onesc_f32[:], 1.0)
            onesc_sb = consts.tile([1, 128], F32R)
            nc.vector.tensor_copy(onesc_sb[:], onesc_f32[:])
            ident_sb = consts.tile([128, 128], BF16)
            make_identity(nc, ident_sb[:])

            xT_sb = consts.tile([128, CI, T], BF16)
            wq_sb = consts.tile([128, CI, NH * D], BF16)
            wk_sb = consts.tile([128, CI, D], BF16)
            wv_sb = consts.tile([128, CI, D], BF16)
            wo_sb = consts.tile([128, NH, C], BF16)
            # (wk,wv,xT) triples feed the first matmuls -- interleave so
            # chunk ci lands as soon as possible; the bulky cos/sin tables
            # ride behind the first two chunks (needed only at rope time),
            # wq behind all xT (consumed from ~38us), wo last.
            for ci in range(CI):
                nc.sync.dma_start(out=wk_sb[:, ci, :], in_=wk_d[_ts(ci, 128), :])
                nc.sync.dma_start(out=wv_sb[:, ci, :], in_=wv_d[_ts(ci, 128), :])
                nc.sync.dma_start(out=xT_sb[:, ci, :], in_=xT_d[_ts(ci, 128), :])
                if ci == 1:
                    nc.sync.dma_start(out=cos_sb[:], in_=cos_d[:])
                    nc.sync.dma_start(out=sin_sb[:], in_=sin_d[:])
            for ci in range(CI):
                nc.sync.dma_start(out=wq_sb[:, ci, :], in_=wq_d[_ts(ci, 128), :])
            for h in range(NH):
                nc.sync.dma_start(out=wo_sb[:, h, :], in_=wo_d[_ts(h, 128), :])

            # ---- persistent activations ---------------------------------
            qT_all = acts.tile([128, NH, T], BF16)   # rotated q^T per head
            kT_all = acts.tile([128, T], BF16)       # rotated k^T
            v_sb = acts.tile([128, TB, D], BF16)     # v in natural [t, d] blocks
            yTn = acts.tile([128, NH, T], BF16)      # normalized y^T per head

            # ---- phase 1: qkv projections + bias + rope + v transpose ---
            # slice pairs: (k, v) first (small weights, streams behind the
            # xT DMAs, ci-outer), then q-head pairs t4-outer so each chunk's
            # bias/rope tail overlaps the remaining matmuls.
            with (
                tc.tile_pool(name="pp", bufs=1, space="PSUM") as pp,
                tc.tile_pool(name="vtp", bufs=2, space="PSUM") as vtp,
                tc.tile_pool(name="rope", bufs=4) as rope_pool,
            ):
                def w_of(m, ci):
                    if m < 4:
                        return wq_sb[:, ci, _ts(m, 128)]
                    if m == 4:
                        return wk_sb[:, ci, :]
                    return wv_sb[:, ci, :]

                def finish_slice(m, t4, ps):
                    """bias + rope (q/k) or transpose (v) for one 512-chunk."""
                    if m == 5:
                        vbb = rope_pool.tile([128, 512], BF16, tag="vbb")
                        nc.vector.tensor_copy(vbb[:], ps[:])
                        for j in range(4):
                            tb = t4 * 4 + j
                            ptv = vtp.tile([128, 128], BF16)
                            nc.tensor.transpose(ptv[:], vbb[:, _ts(j, 128)], ident_sb[:])
                            nc.vector.tensor_copy(v_sb[:, tb, :], ptv[:])
                        return
                    bias_ap = bq_sb[:, m : m + 1] if m < 4 else bk_sb[:, 0:1]
                    qb = rope_pool.tile([128, 512], F32, tag="qb")
                    nc.scalar.activation(qb[:], ps[:], Ident, bias=bias_ap)
                    sh = rope_pool.tile([128, 512], F32, tag="sh")
                    nc.sync.dma_start(out=sh[0:64, :], in_=qb[64:128, :])
                    nc.sync.dma_start(out=sh[64:128, :], in_=qb[0:64, :])
                    t1 = rope_pool.tile([128, 512], F32, tag="t1")
                    nc.vector.tensor_mul(t1[:], qb[:], cos_sb[:, _ts(t4, 512)])
                    nc.vector.tensor_mul(sh[:], sh[:], sin_sb[:, _ts(t4, 512)])
                    dest = (
                        qT_all[:, m, _ts(t4, 512)]
                        if m < 4
                        else kT_all[:, _ts(t4, 512)]
                    )
                    nc.vector.tensor_add(dest, t1[:], sh[:])

                # pair 1: k then v, ci-outer per slice (streams with DMA)
                for m in (4, 5):
                    pss = [
                        pp.tile([128, 512], F32, name=f"pj{m}_{t4}", tag=f"pj{t4}")
                        for t4 in range(TC)
                    ]  # 4 banks, one per t4 tag
                    for ci in range(CI):
                        for t4 in range(TC):
                            nc.tensor.matmul(
                                pss[t4][:],
                                w_of(m, ci),
                                xT_sb[:, ci, _ts(t4, 512)],
                                start=(ci == 0),
                                stop=(ci == CI - 1),
                            )
                    for t4 in range(TC):
                        finish_slice(m, t4, pss[t4])

                # pairs 2-3: q heads, t4-outer (xT fully resident by now)
                for m in (0, 1, 2, 3):
                    for t4 in range(TC):
                        ps = pp.tile([128, 512], F32, tag=f"pj{t4}")
                        for ci in range(CI):
                            nc.tensor.matmul(
                                ps[:],
                                w_of(m, ci),
                                xT_sb[:, ci, _ts(t4, 512)],
                                start=(ci == 0),
                                stop=(ci == CI - 1),
                            )
                        finish_slice(m, t4, ps)

            # ---- phase 2: attention with interleaved o_proj -------------
            if True:
                with (
                    tc.tile_pool(name="st", bufs=2, space="PSUM") as stp,
                    tc.tile_pool(name="yt", bufs=1, space="PSUM") as ytp,
                    tc.tile_pool(name="rsbc", bufs=1, space="PSUM") as rsp,
                    tc.tile_pool(name="po", bufs=2, space="PSUM") as pop,
                    tc.tile_pool(name="ptp", bufs=4) as ptp,
                    tc.tile_pool(name="lg", bufs=2) as lgp,
                    tc.tile_pool(name="inv", bufs=2) as invp,
                    tc.tile_pool(name="rssb", bufs=2) as rssb,
                    tc.tile_pool(name="ytu", bufs=4) as ytup,
                    tc.tile_pool(name="oe", bufs=6) as oep,
                ):
                    def oproj_group(ti, nj, act_evict):
                        ps = pop.tile([128, 512], F32, tag="po")
                        for h in range(NH):
                            nc.tensor.matmul(
                                ps[:],
                                yTn[:, h, _ts(ti, 128)],
                                wo_sb[:, h, _ts(nj, 512)],
                                start=(h == 0),
                                stop=(h == NH - 1),
                            )
                        oe = oep.tile([128, 512], F32)
                        if act_evict:
                            nc.scalar.copy(oe[:], ps[:])
                        else:
                            nc.vector.tensor_copy(oe[:], ps[:])
                        nc.sync.dma_start(
                            out=out_d[_ts(ti, 128), _ts(nj, 512)], in_=oe[:]
                        )

                    pending = []  # o_proj (ti, nj) from the previous qc
                    for qc in (3, 2, 1, 0):
                        nkb = 4 * (qc + 1)
                        iters = NH * (nkb // 2)
                        it = 0
                        emitted = 0
                        for h in range(NH):
                            yt_ps = ytp.tile([128, 512], F32)
                            rs_ps = rsp.tile([1, 512], F32, tag="rsbc")
                            first_kb = nkb - 2  # g runs descending
                            # diagonal-first g order: the tiny trimmed
                            # matmul groups land while the previous head's
                            # norm chain still owns ACT; the full-width
                            # tail keeps PE ahead of exp at the boundary
                            for g in reversed(range(nkb // 2)):
                                st_ps = stp.tile([128, 1024], F32)
                                # trim offsets: j*128 dead cols on diagonal
                                # k-blocks (query index >= key index only)
                                offs = []
                                for u in range(2):
                                    kb = 2 * g + u
                                    j = kb - 4 * qc
                                    offs.append(j * 128 if j > 0 else 0)
                                for u in range(2):
                                    kb = 2 * g + u
                                    o = offs[u]
                                    nc.tensor.matmul(
                                        st_ps[:, u * 512 + o : (u + 1) * 512],
                                        kT_all[:, _ts(kb, 128)],
                                        qT_all[:, h, qc * 512 + o : qc * 512 + 512],
                                        start=True,
                                        stop=True,
                                    )
                                pt = ptp.tile([128, 1024], BF16)
                                nc.scalar.activation(pt[:], st_ps[:], Exp, scale=SCALE)
                                for u in range(2):
                                    kb = 2 * g + u
                                    j = kb - 4 * qc
                                    if j >= 0:  # diagonal block: triangle mask
                                        base = u * 512
                                        blk = pt[:, base + j * 128 : base + (j + 1) * 128]
                                        nc.vector.tensor_mul(blk, blk, tri_sb[:])
                                for u in range(2):
                                    kb = 2 * g + u
                                    o = offs[u]
                                    nc.tensor.matmul(
                                        yt_ps[:, o:512],
                                        v_sb[:, kb, :],
                                        pt[:, u * 512 + o : (u + 1) * 512],
                                        start=(kb == first_kb),
                                        stop=(kb == 1),
                                    )
                                for u in range(2):
                                    kb = 2 * g + u
                                    o = offs[u]
                                    nc.tensor.matmul(
                                        rs_ps[:, o:512],
                                        ones_sb[:],
                                        pt[:, u * 512 + o : (u + 1) * 512],
                                        start=(kb == first_kb),
                                        stop=(kb == 1),
                                    )
                                # spread the previous qc's o_proj over this
                                # qc's attention so PE has fill work during
                                # exp/norm latency (evictions stay on DVE)
                                it += 1
                                want = (len(pending) * it) // iters
                                while emitted < want:
                                    ti, nj = pending[emitted]
                                    oproj_group(ti, nj, act_evict=False)
                                    emitted += 1
                            # evict yT unnormalized right away (frees the
                            # PSUM bank without waiting on the 1/s chain)
                            ytu = ytup.tile([128, 512], BF16)
                            nc.vector.tensor_copy(ytu[:], yt_ps[:])
                            # rowsum row -> SBUF, broadcast to 128 rows with
                            # a K=1 fp32r ones-column matmul (no DRAM trip),
                            # then 1/s = exp(-ln(s)) on the broadcast tile
                            rs_sb = rssb.tile([1, 512], F32R, tag="rs")
                            nc.vector.tensor_copy(rs_sb[:], rs_ps[:])
                            bc_ps = rsp.tile([128, 512], F32, tag="rsbc")
                            nc.tensor.matmul(
                                bc_ps[:], onesc_sb[:], rs_sb[:],
                                start=True, stop=True,
                            )
                            lg = lgp.tile([128, 512], F32)
                            nc.scalar.activation(lg[:], bc_ps[:], Log)
                            inv = invp.tile([128, 512], BF16)
                            nc.scalar.activation(inv[:], lg[:], Exp, scale=-1.0)
                            nc.vector.tensor_mul(
                                yTn[:, h, _ts(qc, 512)], ytu[:], inv[:]
                            )
                        # any leftover interleaved groups, then queue this
                        # qc's own o_proj for the next qc's attention loop
                        for ti, nj in pending[emitted:]:
                            oproj_group(ti, nj, act_evict=False)
                        pending = [
                            (ti, nj)
                            for ti in range(4 * qc, 4 * qc + 4)
                            for nj in range(TC)
                        ]
                    # final qc's o_proj has nothing left to hide behind:
                    # alternate DVE/ACT evictions to drain fastest
                    for i, (ti, nj) in enumerate(pending):
                        oproj_group(ti, nj, act_evict=(i % 2 == 1))

    nc.finalize()
    return nc


def _get_program():
    global _PROGRAM
    if _PROGRAM is None:
        _PROGRAM = _build_program()
    return _PROGRAM


def _rope_tables():
    inv_freq = 1.0 / (1000000.0 ** (np.arange(0, D, 2, dtype=np.float64) / D))
    pos = np.arange(T, dtype=np.float64)
    si = np.outer(pos, inv_freq)                      # [T, D/2]
    cos_h, sin_h = np.cos(si), np.sin(si)
    cos = np.stack([cos_h, cos_h], axis=-1).reshape(T, D)
    sin = np.stack([sin_h, sin_h], axis=-1).reshape(T, D)
    cosT = np.ascontiguousarray(cos.T).astype(np.float32)   # [D, T]
    sinT = np.ascontiguousarray(sin.T).astype(np.float32)
    # rotate-half as a partition shift: sh[i<64]=q[i+64], sh[i>=64]=q[i-64];
    # q_rot = q*cos + sh*sin_signed with the -1 for i<64 baked into the table
    sinT[: D // 2] *= -1.0
    return cosT, sinT


def make_in_maps(x, Wq, bq, Wk, bk, Wv, bv, Wo):
    bf = ml_dtypes.bfloat16
    cosT, sinT = _rope_tables()
    tri = np.triu(np.ones((D, D), dtype=np.float32)).astype(bf)  # [k, q]: q >= k
    in_maps = []
    for b in range(2):
        xT = np.ascontiguousarray(x[b].T).astype(bf)
        for g in range(4):
            in_maps.append(
                {
                    "xT": xT,
                    "wq": np.ascontiguousarray(Wq[:, g * 512 : (g + 1) * 512]).astype(bf),
                    "wk": np.ascontiguousarray(Wk[:, g * 128 : (g + 1) * 128]).astype(bf),
                    "wv": np.ascontiguousarray(Wv[:, g * 128 : (g + 1) * 128]).astype(bf),
                    "wo": np.ascontiguousarray(Wo[g * 512 : (g + 1) * 512, :]).astype(bf),
                    "bq": np.ascontiguousarray(
                        bq[g * 512 : (g + 1) * 512].reshape(NH, D).T
                    ).astype(np.float32),
                    "bk": np.ascontiguousarray(
                        bk[g * 128 : (g + 1) * 128].reshape(D, 1)
                    ).astype(np.float32),
                    "cosT": cosT,
                    "sinT": sinT,
                    "tri": tri,
                }
            )
    return in_maps


def combine_outputs(res, inputs):
    bv, Wo = np.asarray(inputs["bv"]), np.asarray(inputs["Wo"])
    out = np.zeros((2, T, C), dtype=np.float32)
    for c in range(8):
        g = c % 4
        out[c // 4] += res.results[c]["out"]
        # v-bias contribution: softmax rows sum to 1, so bv adds the constant
        # row (bv tiled over the 4 q heads) @ Wo_group to every output row
        bv_tiled = np.tile(bv[g * 128 : (g + 1) * 128], NH).astype(np.float64)
        cvec = bv_tiled @ Wo[g * 512 : (g + 1) * 512, :].astype(np.float64)
        out[c // 4] += cvec.astype(np.float32)[None, :]
    return out


def kernel(x, Wq, bq, Wk, bk, Wv, bv, Wo):
    nc = _get_program()
    in_maps = make_in_maps(x, Wq, bq, Wk, bk, Wv, bv, Wo)
    res = run_bass_kernel_spmd(nc, in_maps, list(range(8)))
    return combine_outputs(res, {"bv": bv, "Wo": Wo})
